# revision 40
# baseline (speedup 1.0000x reference)
"""MultiHeadAttention kernel for 8 Trainium2 NeuronCores (v3).

Reference semantics (direct reshape to [B, H, T, hs] makes "heads" contiguous
256-row blocks of Y.reshape(1536, 64) where Y = x[b] @ W):

    k = (x @ Wk).reshape(B, H, T, hs); q, v likewise
    wei = softmax(mask(q @ k^T * C**-0.5))        (causal over chunk index)
    out = (wei @ v).reshape(B, T, C) @ Wp + bp

Sharding: data-parallel over batch - 16 batches per core, weights replicated,
no collectives.

Final structure (v1 baseline 528 us -> 265 us measured on HW):
  * softmax normalization: 6 denominator rows are DMA-gathered into one
    [6,256] tile, ONE batched DVE reciprocal (v1 spent 159 us in 96
    single-partition reciprocals), broadcast back to [64,256] via a one-hot
    selector matmul (K=6, base partition 0).
  * software pipeline: the tensor engine executes in program order, so the
    next batch's QKV GEMMs are issued BETWEEN the PV matmuls and the
    normalize-dependent (bc/proj) matmuls - the normalize round trip
    (scalar evict -> DMA -> DVE recip) hides under them.
  * PSUM rings decoupled by stage: gemm(pq+pv)=2, att(st0/st1/po)=4,
    bcz(bcp+pz)=2 banks, so cross-batch WAR edges never put next-batch
    GEMMs behind this batch's normalize.
  * e/v/ocT/wp in bf16 (1 cyc/row at any N, cheap gpsimd masks); zq/zk stay
    f32r because bf16 strided evictions are 2x slower on DVE.
  * e1 tiles' left half is permanently zero (startup memset, exp writes only
    cols 128:256): the s-half-1 block is fully masked for t<128.
  * output projection packs head-pairs on 128 partitions (K=128): 6 matmuls.
"""

import sys

if "/opt/trn_rl_repo" not in sys.path:
    sys.path.insert(0, "/opt/trn_rl_repo")

import numpy as np
import ml_dtypes

import concourse.bass as bass
import concourse.mybir as mybir
import concourse.tile as tile
from concourse import bacc
from concourse.bass_utils import run_bass_kernel_spmd

F32 = mybir.dt.float32
F32R = mybir.dt.float32r
BF16 = mybir.dt.bfloat16
Exp = mybir.ActivationFunctionType.Exp

N_CORES = 8
B, T, C = 128, 256, 384
H, HS = 6, 64
NB = B // N_CORES          # batches per core
NP = NB // 2               # batch pairs per core
SCALE = C ** (-0.5)


def build_program(trace_sim=False, sim_init=False):
    nc = bacc.Bacc("TRN2", target_bir_lowering=False, debug=False)

    xT_d = nc.dram_tensor("xT", [NB, 3, 128, T], F32R, kind="ExternalInput")
    # qkv weights host-packed to [128, 3, C] (partition-major, contiguous)
    wq_d = nc.dram_tensor("wqp", [128, 3, C], F32R, kind="ExternalInput")
    wk_d = nc.dram_tensor("wkp", [128, 3, C], F32R, kind="ExternalInput")
    wv_d = nc.dram_tensor("wvp", [128, 3, C], F32R, kind="ExternalInput")
    # Wp pre-packed on host: wp2[64*par + d, jp, c] = Wp[(2*jp+par)*64 + d, c]
    wp_d = nc.dram_tensor("wp2", [128, 3, C], BF16, kind="ExternalInput")
    bpb_d = nc.dram_tensor("bpb", [128, C], F32, kind="ExternalInput")
    tri_d = nc.dram_tensor("tri", [128, 128], BF16, kind="ExternalInput")
    sel_d = nc.dram_tensor("sel6", [6, C], BF16, kind="ExternalInput")
    vsc_d = nc.dram_tensor("vsc", [NB, T, C], BF16)
    out_d = nc.dram_tensor("out", [NB, T, C], F32, kind="ExternalOutput")

    with tile.TileContext(nc, trace_sim=trace_sim) as tc:
        with (
            tc.tile_pool(name="const", bufs=1) as cst,
            tc.tile_pool(name="xt", bufs=3) as xtp,
            tc.tile_pool(name="zqk", bufs=2) as zqkp,
            tc.tile_pool(name="yv", bufs=4) as yvp,
            tc.tile_pool(name="vsb", bufs=3) as vp,
            tc.tile_pool(name="ee", bufs=6) as ep,
            tc.tile_pool(name="o65", bufs=14) as op65,
            tc.tile_pool(name="rs", bufs=2) as rsp,
            tc.tile_pool(name="oct", bufs=2) as octp,
            tc.tile_pool(name="zo", bufs=3) as zop,
            tc.tile_pool(name="psG", bufs=2, space="PSUM") as psG,
            tc.tile_pool(name="psA", bufs=4, space="PSUM") as psA,
            tc.tile_pool(name="psZ", bufs=2, space="PSUM") as psZ,
        ):
            # ---- constants (Q/K weights stream first so the prologue
            # GEMMs can start while the rest of the constants load) ----
            wq_sb = cst.tile([128, 3, C], F32R, tag="wq")
            wk_sb = cst.tile([128, 3, C], F32R, tag="wk")
            wv_sb = cst.tile([128, 3, C], F32R, tag="wv")
            for wsb, wd in ((wq_sb, wq_d), (wk_sb, wk_d)):
                nc.sync.dma_start(wsb[:], wd[:])
            wp_sb = cst.tile([128, 3, C], BF16, tag="wp")
            bpb = cst.tile([128, C], F32, tag="bpb")
            tri = cst.tile([128, 128], BF16, tag="tri")
            sel6 = cst.tile([6, C], BF16, tag="sel6")

            def load_tail_consts():
                nc.sync.dma_start(wv_sb[:], wv_d[:])
                nc.sync.dma_start(wp_sb[:], wp_d[:])
                nc.sync.dma_start(bpb[:], bpb_d[:])
                nc.sync.dma_start(tri[:], tri_d[:])
                nc.sync.dma_start(sel6[:], sel_d[:])

            # Pre-zero the e1 ring (left half stays zero forever: exp only
            # writes cols 128:256) and pre-set the V ones columns per slot.
            for _ in range(6):
                e1i = ep.tile([128, T], BF16, tag="e1")
                nc.vector.memset(e1i[:], 0.0)
            for _ in range(3):
                v_ini = vp.tile([128, 12 * (HS + 1)], BF16, tag="vsb")
                if sim_init:
                    nc.vector.memset(v_ini[:], 0.0)
                nc.vector.memset(
                    v_ini[:].rearrange("p (g d) -> p g d", d=HS + 1)[:, :, HS], 1.0)

            # ---------------- pipeline building blocks ----------------
            xts = {}
            zqks = {}
            vsbs = {}

            def load_xt(p):
                xt = xtp.tile([128, 3, 2 * T], F32R, tag="xt")
                for n in range(2):
                    nc.sync.dma_start(
                        xt[:].rearrange("p k (n t) -> p k n t", n=2)[:, :, n, :],
                        xT_d[2 * p + n].rearrange("k p t -> p k t"))
                xts[p] = xt

            def qk_group(p, wi, m):
                # one (weight, m-tile) slice of a batch pair's Q/K GEMMs
                if p not in zqks:
                    zq2 = zqkp.tile([64, 12 * T], BF16, tag="zq")
                    zk2 = zqkp.tile([64, 12 * T], BF16, tag="zk")
                    if sim_init:
                        nc.vector.memset(zq2[:], 0.0)
                        nc.vector.memset(zk2[:], 0.0)
                    zqks[p] = (zq2, zk2)
                xt = xts[p]
                wsb = (wq_sb, wk_sb)[wi]
                z2 = zqks[p][wi]
                pq = psG.tile([128, 2 * T], F32, tag="gemm")
                for k in range(3):
                    nc.tensor.matmul(
                        pq[:],
                        wsb[:, k, m * 128:(m + 1) * 128],
                        xt[:, k, :],
                        start=(k == 0), stop=(k == 2),
                    )
                # eviction: [d, bb*1536 + 6t + j] for bb in {0,1}
                nc.vector.tensor_copy(
                    z2[:].rearrange("d (n f) -> d n f", n=2)
                         [:, :, 2 * m:6 * T:6]
                         .rearrange("d n t -> d (n t)"),
                    pq[0:64, :])
                nc.scalar.copy(
                    z2[:].rearrange("d (n f) -> d n f", n=2)
                         [:, :, 2 * m + 1:6 * T:6]
                         .rearrange("d n t -> d (n t)"),
                    pq[64:128, :])

            def qk_pair(p):
                for wi in range(2):
                    for m in range(3):
                        qk_group(p, wi, m)

            def v_group(b, m):
                xt = xts[b // 2]
                pv = psG.tile([128, C], F32, tag="gemm")
                for k in range(3):
                    nc.tensor.matmul(
                        pv[:],
                        xt[:, k, (b % 2) * T + m * 128:
                           (b % 2) * T + (m + 1) * 128],
                        wv_sb[:, k, :],
                        start=(k == 0), stop=(k == 2),
                    )
                yv = yvp.tile([128, C], BF16, tag="yv")
                nc.vector.tensor_copy(yv[:], pv[:])
                nc.sync.dma_start(vsc_d[b, m * 128:(m + 1) * 128, :], yv[:])
                if m == 1:
                    # V in chunk-row layout [128, 12*(64+1)]; ones cols preset
                    v_sb = vp.tile([128, 12 * (HS + 1)], BF16, tag="vsb")
                    nc.sync.dma_start(
                        v_sb[:].rearrange("p (g d) -> p g d", d=HS + 1)
                              [:, :, 0:HS],
                        vsc_d[b].rearrange("t c -> (t c)")
                                .rearrange("(g p d) -> p g d", p=128, d=64),
                    )
                    vsbs[b] = v_sb

            def v_batch(b):
                v_group(b, 0)
                v_group(b, 1)

            def attention(b, fillers=()):
                zq2, zk2 = zqks[b // 2]
                zq = zq2[:, (b % 2) * 6 * T:(b % 2 + 1) * 6 * T]
                zk = zk2[:, (b % 2) * 6 * T:(b % 2 + 1) * 6 * T]
                v_sb = vsbs.pop(b)
                rs6 = rsp.tile([6, T], BF16, tag="rs6")
                o65_tiles = []

                def s_head(h):
                    qs = zq[:, h * T:(h + 1) * T]
                    # s-half 0: mask is upper-tri on cols 0:128, open after
                    st0 = psA.tile([128, T], F32, tag="att")
                    nc.tensor.matmul(
                        st0[:], zk[:, h * T:h * T + 128], qs,
                        start=True, stop=True)
                    e0 = ep.tile([128, T], BF16, tag="e0")
                    nc.scalar.activation(e0[:], st0[:], Exp,
                                         scale=SCALE)
                    nc.gpsimd.tensor_mul(e0[:, 0:128], e0[:, 0:128], tri[:])
                    # s-half 1: t<128 fully masked (e1 left half stays
                    # zero), so only the visible N=128 block is computed
                    # (bf16 matmul runs 1 cyc/row at any free dim)
                    st1 = psA.tile([128, 128], F32, tag="att")
                    nc.tensor.matmul(
                        st1[:], zk[:, h * T + 128:h * T + 256],
                        qs[:, 128:256],
                        start=True, stop=True)
                    e1 = ep.tile([128, T], BF16, tag="e1")
                    nc.scalar.activation(e1[:, 128:256], st1[:], Exp,
                                         scale=SCALE)
                    nc.gpsimd.tensor_mul(e1[:, 128:256], e1[:, 128:256],
                                         tri[:])
                    return e0, e1

                # S runs one head ahead of PV so each head's exp->mask chain
                # drains while the next head's S matmuls (and a filler GEMM
                # group) stream through the tensor engine
                es = [s_head(0)]
                for h in range(H):
                    if h + 1 < H:
                        es.append(s_head(h + 1))
                    if h < len(fillers):
                        fillers[h]()
                    e0, e1 = es[h]
                    # PV with fused rowsum (row 64 of each lhsT chunk is ones)
                    po = psA.tile([HS + 1, T], F32, tag="att")
                    nc.tensor.matmul(
                        po[:],
                        v_sb[:, (2 * h) * (HS + 1):(2 * h + 1) * (HS + 1)],
                        e0[:], start=True, stop=False)
                    nc.tensor.matmul(
                        po[:],
                        v_sb[:, (2 * h + 1) * (HS + 1):(2 * h + 2) * (HS + 1)],
                        e1[:], start=False, stop=True)
                    # evict now (frees the PSUM slot): rows 0:64 = O^T, row 64
                    # = softmax denominator
                    o65 = op65.tile([HS + 1, T], BF16, tag="o65")
                    nc.scalar.copy(o65[:], po[:])
                    o65_tiles.append(o65)
                    # gather this head's denominators into rs6 row h (DMA:
                    # engine copies cannot write partitions not 32-aligned)
                    nc.sync.dma_start(rs6[h:h + 1, :], o65[HS:HS + 1, :])
                for f in fillers[H:]:
                    f()
                return rs6, o65_tiles

            def norm_pieces(b, rs6, o65_tiles):
                # normalize + projection, decomposed into five tensor-work
                # chunks so they can double as fillers for the last batch
                st = {}

                def bc_piece(hp):
                    if "recip6" not in st:
                        recip6 = rsp.tile([6, T], BF16, tag="recip6")
                        with nc.allow_low_precision(
                                reason="softmax denom in bf16"):
                            nc.vector.reciprocal(recip6[:], rs6[:])
                        # ocT2[64*(g%2) + d, g//2] = O^T[d, g] / denom[g]
                        st["recip6"] = recip6
                        st["ocT2"] = octp.tile([128, 768], BF16, tag="ocT2",
                                               name="ocT2")
                    # bc broadcasts a head PAIR per matmul: out rows 0:64 =
                    # head 2hp's denominators, rows 64:128 = head 2hp+1's
                    bcp = psZ.tile([128, T], F32, tag="bcz")
                    nc.tensor.matmul(
                        bcp[:], sel6[:, 128 * hp:128 * (hp + 1)],
                        st["recip6"][:], start=True, stop=True)
                    for hh in range(2):
                        h = 2 * hp + hh
                        o65 = o65_tiles[h]
                        for par in range(2):
                            nc.vector.tensor_mul(
                                st["ocT2"][64 * par:64 * (par + 1),
                                           128 * h:128 * (h + 1)],
                                o65[0:HS, par::2],
                                bcp[64 * hh:64 * (hh + 1), par::2])

                def proj_piece(m):
                    pz = psZ.tile([128, C], F32, tag="bcz")
                    for jp in range(3):
                        nc.tensor.matmul(
                            pz[:],
                            st["ocT2"][:, 384 * m + jp:384 * (m + 1):3],
                            wp_sb[:, jp, :],
                            start=(jp == 0), stop=(jp == 2),
                        )
                    zo = zop.tile([128, C], F32, tag="zo")
                    nc.vector.tensor_add(zo[:], pz[:], bpb[:])
                    nc.sync.dma_start(out_d[b, m * 128:(m + 1) * 128, :], zo[:])

                return ([lambda hp=hp: bc_piece(hp) for hp in range(H // 2)]
                        + [lambda m=m: proj_piece(m) for m in range(2)])

            def norm_proj(b, rs6, o65_tiles):
                for f in norm_pieces(b, rs6, o65_tiles):
                    f()

            # ---------------- software-pipelined schedule ----------------
            # Gap work (next-batch GEMMs) sits between PV(b) and the
            # normalize-dependent bc/proj matmuls so the tensor queue never
            # drains while the scalar->DMA->DVE reciprocal round trip runs.
            load_xt(0)
            load_tail_consts()
            qk_pair(0)
            v_batch(0)
            v_batch(1)
            load_xt(1)
            prev = None
            for b in range(NB):
                fillers = []
                if b % 2 == 0:
                    if b // 2 + 1 < NP:
                        p = b // 2 + 1
                        for wi in range(2):
                            for m in range(3):
                                fillers.append(
                                    lambda p=p, wi=wi, m=m: qk_group(p, wi, m))
                else:
                    for bn in (b + 1, b + 2):
                        if bn < NB:
                            for m in range(2):
                                fillers.append(
                                    lambda bn=bn, m=m: v_group(bn, m))
                    if b // 2 + 2 < NP:
                        fillers.append(lambda p=b // 2 + 2: load_xt(p))
                # normalize+project run one batch behind: by the time bc(b-1)
                # reaches the tensor queue, recip(b-1) completed under
                # attention(b)'s shadow. For the last batch (no next-batch
                # GEMMs left) the previous batch's norm/proj pieces are the
                # fillers; slots 0-1 stay empty so recip(b-1) has time.
                if b == NB - 1 and prev is not None:
                    fillers = [lambda: None, lambda: None]
                    fillers += norm_pieces(b - 1, *prev)
                    prev = None
                cur = attention(b, fillers)
                if prev is not None:
                    norm_proj(b - 1, *prev)
                prev = cur
            norm_proj(NB - 1, *prev)

    nc.compile()
    return nc


def make_in_maps(x, Wk, Wq, Wv, Wp, bp):
    ut = (np.arange(128)[:, None] <= np.arange(128)[None, :])
    tri = ut.astype(ml_dtypes.bfloat16)
    sel6 = np.zeros((6, C), np.float32)
    for h in range(6):
        sel6[h, 64 * h:64 * (h + 1)] = 1.0
    wp2 = (np.asarray(Wp, np.float32).reshape(3, 2, 64, C)
           .transpose(1, 2, 0, 3).reshape(128, 3, C))
    def pack_w(W):
        return np.ascontiguousarray(
            np.asarray(W, np.float32).reshape(3, 128, C).transpose(1, 0, 2))

    common = dict(
        wqp=pack_w(Wq),
        wkp=pack_w(Wk),
        wvp=pack_w(Wv),
        wp2=np.ascontiguousarray(wp2).astype(ml_dtypes.bfloat16),
        bpb=np.broadcast_to(np.asarray(bp, np.float32), (128, C)).copy(),
        tri=np.ascontiguousarray(tri),
        sel6=sel6.astype(ml_dtypes.bfloat16),
    )
    in_maps = []
    for c in range(N_CORES):
        xs = np.asarray(x[c * NB:(c + 1) * NB], np.float32)
        xT = np.ascontiguousarray(xs.transpose(0, 2, 1)).reshape(NB, 3, 128, T)
        in_maps.append(dict(common, xT=xT))
    return in_maps


_CACHE = {}


def kernel(x, Wk, Wq, Wv, Wp, bp, _trace=False, _tmpdir=None):
    if "nc" not in _CACHE:
        _CACHE["nc"] = build_program()
    nc = _CACHE["nc"]
    in_maps = make_in_maps(x, Wk, Wq, Wv, Wp, bp)
    res = run_bass_kernel_spmd(nc, in_maps, list(range(N_CORES)),
                               trace=_trace, tmpdir=_tmpdir)
    _CACHE["last_results"] = res
    out = np.concatenate([np.asarray(r["out"]) for r in res.results], axis=0)
    return out.reshape(B, T, C).astype(np.float32)


# revision 41
# speedup vs baseline: 1.1876x; 1.1876x over previous
"""MultiHeadAttention kernel for 8 Trainium2 NeuronCores (v3).

Reference semantics (direct reshape to [B, H, T, hs] makes "heads" contiguous
256-row blocks of Y.reshape(1536, 64) where Y = x[b] @ W):

    k = (x @ Wk).reshape(B, H, T, hs); q, v likewise
    wei = softmax(mask(q @ k^T * C**-0.5))        (causal over chunk index)
    out = (wei @ v).reshape(B, T, C) @ Wp + bp

Sharding: data-parallel over batch - 16 batches per core, weights replicated,
no collectives.

Final structure (v1 baseline 528 us -> 265-276 us measured on HW):
  * softmax normalization: 6 denominator rows are DMA-gathered into one
    [6,256] tile, ONE batched DVE reciprocal (v1 spent 159 us in 96
    single-partition reciprocals), broadcast back to [64,256] via a one-hot
    selector matmul (K=6, base partition 0).
  * software pipeline: the tensor engine executes in program order, so the
    next batch's QKV GEMMs are issued BETWEEN the PV matmuls and the
    normalize-dependent (bc/proj) matmuls - the normalize round trip
    (scalar evict -> DMA -> DVE recip) hides under them.
  * PSUM rings decoupled by stage: gemm(pq+pv)=2, att(st0/st1/po)=4,
    bcz(bcp+pz)=2 banks, so cross-batch WAR edges never put next-batch
    GEMMs behind this batch's normalize.
  * e/v/ocT/wp in bf16 (1 cyc/row at any N, cheap gpsimd masks); zq/zk stay
    f32r because bf16 strided evictions are 2x slower on DVE.
  * e1 tiles' left half is permanently zero (startup memset, exp writes only
    cols 128:256): the s-half-1 block is fully masked for t<128.
  * output projection packs head-pairs on 128 partitions (K=128): 6 matmuls.
"""

import sys

if "/opt/trn_rl_repo" not in sys.path:
    sys.path.insert(0, "/opt/trn_rl_repo")

import numpy as np
import ml_dtypes

import concourse.bass as bass
import concourse.mybir as mybir
import concourse.tile as tile
from concourse import bacc
from concourse.bass_utils import run_bass_kernel_spmd

F32 = mybir.dt.float32
F32R = mybir.dt.float32r
BF16 = mybir.dt.bfloat16
Exp = mybir.ActivationFunctionType.Exp

N_CORES = 8
B, T, C = 128, 256, 384
H, HS = 6, 64
NB = B // N_CORES          # batches per core
NP = NB // 2               # batch pairs per core
SCALE = C ** (-0.5)


def build_program(trace_sim=False, sim_init=False):
    nc = bacc.Bacc("TRN2", target_bir_lowering=False, debug=False)

    xT_d = nc.dram_tensor("xT", [NB, 3, 128, T], F32R, kind="ExternalInput")
    wq_d = nc.dram_tensor("wq", [C, C], F32R, kind="ExternalInput")
    wk_d = nc.dram_tensor("wk", [C, C], F32R, kind="ExternalInput")
    wv_d = nc.dram_tensor("wv", [C, C], F32R, kind="ExternalInput")
    # Wp pre-packed on host: wp2[64*par + d, jp, c] = Wp[(2*jp+par)*64 + d, c]
    wp_d = nc.dram_tensor("wp2", [128, 3, C], BF16, kind="ExternalInput")
    bpb_d = nc.dram_tensor("bpb", [128, C], F32, kind="ExternalInput")
    tri_d = nc.dram_tensor("tri", [128, 128], BF16, kind="ExternalInput")
    sel_d = nc.dram_tensor("sel6", [6, C], BF16, kind="ExternalInput")
    vsc_d = nc.dram_tensor("vsc", [NB, T, C], BF16)
    out_d = nc.dram_tensor("out", [NB, T, C], F32, kind="ExternalOutput")

    with tile.TileContext(nc, trace_sim=trace_sim) as tc:
        with (
            tc.tile_pool(name="const", bufs=1) as cst,
            tc.tile_pool(name="xt", bufs=3) as xtp,
            tc.tile_pool(name="zqk", bufs=2) as zqkp,
            tc.tile_pool(name="yv", bufs=4) as yvp,
            tc.tile_pool(name="vsb", bufs=3) as vp,
            tc.tile_pool(name="ee", bufs=6) as ep,
            tc.tile_pool(name="o65", bufs=14) as op65,
            tc.tile_pool(name="rs", bufs=2) as rsp,
            tc.tile_pool(name="oct", bufs=2) as octp,
            tc.tile_pool(name="zo", bufs=3) as zop,
            tc.tile_pool(name="psG", bufs=2, space="PSUM") as psG,
            tc.tile_pool(name="psA", bufs=4, space="PSUM") as psA,
            tc.tile_pool(name="psZ", bufs=2, space="PSUM") as psZ,
        ):
            # ---- constants (Q/K weights stream first so the prologue
            # GEMMs can start while the rest of the constants load) ----
            wq_sb = cst.tile([128, 3, C], F32R, tag="wq")
            wk_sb = cst.tile([128, 3, C], F32R, tag="wk")
            wv_sb = cst.tile([128, 3, C], F32R, tag="wv")
            for wsb, wd in ((wq_sb, wq_d), (wk_sb, wk_d)):
                nc.sync.dma_start(wsb[:], wd.rearrange("(k p) c -> p k c", p=128))
            wp_sb = cst.tile([128, 3, C], BF16, tag="wp")
            bpb = cst.tile([128, C], F32, tag="bpb")
            tri = cst.tile([128, 128], BF16, tag="tri")
            sel6 = cst.tile([6, C], BF16, tag="sel6")

            def load_tail_consts():
                nc.sync.dma_start(
                    wv_sb[:], wv_d.rearrange("(k p) c -> p k c", p=128))
                nc.sync.dma_start(wp_sb[:], wp_d[:])
                nc.sync.dma_start(bpb[:], bpb_d[:])
                nc.sync.dma_start(tri[:], tri_d[:])
                nc.sync.dma_start(sel6[:], sel_d[:])

            # Pre-zero the e1 ring (left half stays zero forever: exp only
            # writes cols 128:256) and pre-set the V ones columns per slot.
            for _ in range(6):
                e1i = ep.tile([128, T], BF16, tag="e1")
                nc.vector.memset(e1i[:], 0.0)
            for _ in range(3):
                v_ini = vp.tile([128, 12 * (HS + 1)], BF16, tag="vsb")
                if sim_init:
                    nc.vector.memset(v_ini[:], 0.0)
                nc.vector.memset(
                    v_ini[:].rearrange("p (g d) -> p g d", d=HS + 1)[:, :, HS], 1.0)

            # ---------------- pipeline building blocks ----------------
            xts = {}
            zqks = {}
            vsbs = {}

            def load_xt(p):
                xt = xtp.tile([128, 3, 2 * T], F32R, tag="xt")
                for n in range(2):
                    nc.sync.dma_start(
                        xt[:].rearrange("p k (n t) -> p k n t", n=2)[:, :, n, :],
                        xT_d[2 * p + n].rearrange("k p t -> p k t"))
                xts[p] = xt

            def qk_group(p, wi, m):
                # one (weight, m-tile) slice of a batch pair's Q/K GEMMs
                if p not in zqks:
                    zq2 = zqkp.tile([64, 12 * T], F32R, tag="zq")
                    zk2 = zqkp.tile([64, 12 * T], F32R, tag="zk")
                    if sim_init:
                        nc.vector.memset(zq2[:], 0.0)
                        nc.vector.memset(zk2[:], 0.0)
                    zqks[p] = (zq2, zk2)
                xt = xts[p]
                wsb = (wq_sb, wk_sb)[wi]
                z2 = zqks[p][wi]
                pq = psG.tile([128, 2 * T], F32, tag="gemm")
                for k in range(3):
                    nc.tensor.matmul(
                        pq[:],
                        wsb[:, k, m * 128:(m + 1) * 128],
                        xt[:, k, :],
                        start=(k == 0), stop=(k == 2),
                    )
                # eviction: [d, bb*1536 + 6t + j] for bb in {0,1}
                nc.vector.tensor_copy(
                    z2[:].rearrange("d (n f) -> d n f", n=2)
                         [:, :, 2 * m:6 * T:6]
                         .rearrange("d n t -> d (n t)"),
                    pq[0:64, :])
                nc.scalar.copy(
                    z2[:].rearrange("d (n f) -> d n f", n=2)
                         [:, :, 2 * m + 1:6 * T:6]
                         .rearrange("d n t -> d (n t)"),
                    pq[64:128, :])

            def qk_pair(p):
                for wi in range(2):
                    for m in range(3):
                        qk_group(p, wi, m)

            def v_group(b, m):
                xt = xts[b // 2]
                pv = psG.tile([128, C], F32, tag="gemm")
                for k in range(3):
                    nc.tensor.matmul(
                        pv[:],
                        xt[:, k, (b % 2) * T + m * 128:
                           (b % 2) * T + (m + 1) * 128],
                        wv_sb[:, k, :],
                        start=(k == 0), stop=(k == 2),
                    )
                yv = yvp.tile([128, C], BF16, tag="yv")
                nc.vector.tensor_copy(yv[:], pv[:])
                nc.sync.dma_start(vsc_d[b, m * 128:(m + 1) * 128, :], yv[:])
                if m == 1:
                    # V in chunk-row layout [128, 12*(64+1)]; ones cols preset
                    v_sb = vp.tile([128, 12 * (HS + 1)], BF16, tag="vsb")
                    nc.sync.dma_start(
                        v_sb[:].rearrange("p (g d) -> p g d", d=HS + 1)
                              [:, :, 0:HS],
                        vsc_d[b].rearrange("t c -> (t c)")
                                .rearrange("(g p d) -> p g d", p=128, d=64),
                    )
                    vsbs[b] = v_sb

            def v_batch(b):
                v_group(b, 0)
                v_group(b, 1)

            def attention(b, fillers=()):
                zq2, zk2 = zqks[b // 2]
                zq = zq2[:, (b % 2) * 6 * T:(b % 2 + 1) * 6 * T]
                zk = zk2[:, (b % 2) * 6 * T:(b % 2 + 1) * 6 * T]
                v_sb = vsbs.pop(b)
                rs6 = rsp.tile([6, T], BF16, tag="rs6")
                o65_tiles = []

                def s_head(h):
                    qs = zq[:, h * T:(h + 1) * T]
                    # s-half 0: mask is upper-tri on cols 0:128, open after
                    st0 = psA.tile([128, T], F32, tag="att")
                    nc.tensor.matmul(
                        st0[:], zk[:, h * T:h * T + 128], qs,
                        start=True, stop=True)
                    e0 = ep.tile([128, T], BF16, tag="e0")
                    nc.scalar.activation(e0[:], st0[:], Exp,
                                         scale=SCALE)
                    nc.gpsimd.tensor_mul(e0[:, 0:128], e0[:, 0:128], tri[:])
                    # s-half 1: t<128 fully masked (e1 left half stays zero);
                    # cols 0:128 of st1 are computed but never read (f32r
                    # matmuls need free dim >= 256 for full rate anyway)
                    st1 = psA.tile([128, T], F32, tag="att")
                    nc.tensor.matmul(
                        st1[:], zk[:, h * T + 128:h * T + 256], qs,
                        start=True, stop=True)
                    e1 = ep.tile([128, T], BF16, tag="e1")
                    nc.scalar.activation(e1[:, 128:256], st1[:, 128:256], Exp,
                                         scale=SCALE)
                    nc.gpsimd.tensor_mul(e1[:, 128:256], e1[:, 128:256],
                                         tri[:])
                    return e0, e1

                # S runs one head ahead of PV so each head's exp->mask chain
                # drains while the next head's S matmuls (and a filler GEMM
                # group) stream through the tensor engine
                es = [s_head(0)]
                for h in range(H):
                    if h + 1 < H:
                        es.append(s_head(h + 1))
                    if h < len(fillers):
                        fillers[h]()
                    e0, e1 = es[h]
                    # PV with fused rowsum (row 64 of each lhsT chunk is ones)
                    po = psA.tile([HS + 1, T], F32, tag="att")
                    nc.tensor.matmul(
                        po[:],
                        v_sb[:, (2 * h) * (HS + 1):(2 * h + 1) * (HS + 1)],
                        e0[:], start=True, stop=False)
                    nc.tensor.matmul(
                        po[:],
                        v_sb[:, (2 * h + 1) * (HS + 1):(2 * h + 2) * (HS + 1)],
                        e1[:], start=False, stop=True)
                    # evict now (frees the PSUM slot): rows 0:64 = O^T, row 64
                    # = softmax denominator
                    o65 = op65.tile([HS + 1, T], BF16, tag="o65")
                    nc.scalar.copy(o65[:], po[:])
                    o65_tiles.append(o65)
                    # gather this head's denominators into rs6 row h (DMA:
                    # engine copies cannot write partitions not 32-aligned)
                    nc.sync.dma_start(rs6[h:h + 1, :], o65[HS:HS + 1, :])
                for f in fillers[H:]:
                    f()
                return rs6, o65_tiles

            def norm_pieces(b, rs6, o65_tiles):
                # normalize + projection, decomposed into five tensor-work
                # chunks so they can double as fillers for the last batch
                st = {}

                def bc_piece(hp):
                    if "recip6" not in st:
                        recip6 = rsp.tile([6, T], BF16, tag="recip6")
                        with nc.allow_low_precision(
                                reason="softmax denom in bf16"):
                            nc.vector.reciprocal(recip6[:], rs6[:])
                        # ocT2[64*(g%2) + d, g//2] = O^T[d, g] / denom[g]
                        st["recip6"] = recip6
                        st["ocT2"] = octp.tile([128, 768], BF16, tag="ocT2",
                                               name="ocT2")
                    # bc broadcasts a head PAIR per matmul: out rows 0:64 =
                    # head 2hp's denominators, rows 64:128 = head 2hp+1's
                    bcp = psZ.tile([128, T], F32, tag="bcz")
                    nc.tensor.matmul(
                        bcp[:], sel6[:, 128 * hp:128 * (hp + 1)],
                        st["recip6"][:], start=True, stop=True)
                    for hh in range(2):
                        h = 2 * hp + hh
                        o65 = o65_tiles[h]
                        for par in range(2):
                            nc.vector.tensor_mul(
                                st["ocT2"][64 * par:64 * (par + 1),
                                           128 * h:128 * (h + 1)],
                                o65[0:HS, par::2],
                                bcp[64 * hh:64 * (hh + 1), par::2])

                def proj_piece(m):
                    pz = psZ.tile([128, C], F32, tag="bcz")
                    for jp in range(3):
                        nc.tensor.matmul(
                            pz[:],
                            st["ocT2"][:, 384 * m + jp:384 * (m + 1):3],
                            wp_sb[:, jp, :],
                            start=(jp == 0), stop=(jp == 2),
                        )
                    zo = zop.tile([128, C], F32, tag="zo")
                    nc.vector.tensor_add(zo[:], pz[:], bpb[:])
                    nc.sync.dma_start(out_d[b, m * 128:(m + 1) * 128, :], zo[:])

                return ([lambda hp=hp: bc_piece(hp) for hp in range(H // 2)]
                        + [lambda m=m: proj_piece(m) for m in range(2)])

            def norm_proj(b, rs6, o65_tiles):
                for f in norm_pieces(b, rs6, o65_tiles):
                    f()

            # ---------------- software-pipelined schedule ----------------
            # Gap work (next-batch GEMMs) sits between PV(b) and the
            # normalize-dependent bc/proj matmuls so the tensor queue never
            # drains while the scalar->DMA->DVE reciprocal round trip runs.
            load_xt(0)
            load_tail_consts()
            qk_pair(0)
            v_batch(0)
            v_batch(1)
            load_xt(1)
            prev = None
            for b in range(NB):
                fillers = []
                if b % 2 == 0:
                    if b // 2 + 1 < NP:
                        p = b // 2 + 1
                        for wi in range(2):
                            for m in range(3):
                                fillers.append(
                                    lambda p=p, wi=wi, m=m: qk_group(p, wi, m))
                else:
                    for bn in (b + 1, b + 2):
                        if bn < NB:
                            for m in range(2):
                                fillers.append(
                                    lambda bn=bn, m=m: v_group(bn, m))
                    if b // 2 + 2 < NP:
                        fillers.append(lambda p=b // 2 + 2: load_xt(p))
                # normalize+project run one batch behind: by the time bc(b-1)
                # reaches the tensor queue, recip(b-1) completed under
                # attention(b)'s shadow. For the last batch (no next-batch
                # GEMMs left) the previous batch's norm/proj pieces are the
                # fillers; slots 0-1 stay empty so recip(b-1) has time.
                if b == NB - 1 and prev is not None:
                    fillers = [lambda: None, lambda: None]
                    fillers += norm_pieces(b - 1, *prev)
                    prev = None
                cur = attention(b, fillers)
                if prev is not None:
                    norm_proj(b - 1, *prev)
                prev = cur
            norm_proj(NB - 1, *prev)

    nc.compile()
    return nc


def make_in_maps(x, Wk, Wq, Wv, Wp, bp):
    ut = (np.arange(128)[:, None] <= np.arange(128)[None, :])
    tri = ut.astype(ml_dtypes.bfloat16)
    sel6 = np.zeros((6, C), np.float32)
    for h in range(6):
        sel6[h, 64 * h:64 * (h + 1)] = 1.0
    wp2 = (np.asarray(Wp, np.float32).reshape(3, 2, 64, C)
           .transpose(1, 2, 0, 3).reshape(128, 3, C))
    common = dict(
        wq=np.ascontiguousarray(Wq, np.float32),
        wk=np.ascontiguousarray(Wk, np.float32),
        wv=np.ascontiguousarray(Wv, np.float32),
        wp2=np.ascontiguousarray(wp2).astype(ml_dtypes.bfloat16),
        bpb=np.broadcast_to(np.asarray(bp, np.float32), (128, C)).copy(),
        tri=np.ascontiguousarray(tri),
        sel6=sel6.astype(ml_dtypes.bfloat16),
    )
    in_maps = []
    for c in range(N_CORES):
        xs = np.asarray(x[c * NB:(c + 1) * NB], np.float32)
        xT = np.ascontiguousarray(xs.transpose(0, 2, 1)).reshape(NB, 3, 128, T)
        in_maps.append(dict(common, xT=xT))
    return in_maps


_CACHE = {}


def kernel(x, Wk, Wq, Wv, Wp, bp, _trace=False, _tmpdir=None):
    if "nc" not in _CACHE:
        _CACHE["nc"] = build_program()
    nc = _CACHE["nc"]
    in_maps = make_in_maps(x, Wk, Wq, Wv, Wp, bp)
    res = run_bass_kernel_spmd(nc, in_maps, list(range(N_CORES)),
                               trace=_trace, tmpdir=_tmpdir)
    _CACHE["last_results"] = res
    out = np.concatenate([np.asarray(r["out"]) for r in res.results], axis=0)
    return out.reshape(B, T, C).astype(np.float32)


# revision 42
# speedup vs baseline: 1.2257x; 1.0321x over previous
"""MultiHeadAttention kernel for 8 Trainium2 NeuronCores (v3).

Reference semantics (direct reshape to [B, H, T, hs] makes "heads" contiguous
256-row blocks of Y.reshape(1536, 64) where Y = x[b] @ W):

    k = (x @ Wk).reshape(B, H, T, hs); q, v likewise
    wei = softmax(mask(q @ k^T * C**-0.5))        (causal over chunk index)
    out = (wei @ v).reshape(B, T, C) @ Wp + bp

Sharding: data-parallel over batch - 16 batches per core, weights replicated,
no collectives.

Final structure (v1 baseline 528 us -> 265-276 us measured on HW):
  * softmax normalization: 6 denominator rows are DMA-gathered into one
    [6,256] tile, ONE batched DVE reciprocal (v1 spent 159 us in 96
    single-partition reciprocals), broadcast back to [64,256] via a one-hot
    selector matmul (K=6, base partition 0).
  * software pipeline: the tensor engine executes in program order, so the
    next batch's QKV GEMMs are issued BETWEEN the PV matmuls and the
    normalize-dependent (bc/proj) matmuls - the normalize round trip
    (scalar evict -> DMA -> DVE recip) hides under them.
  * PSUM rings decoupled by stage: gemm(pq+pv)=2, att(st0/st1/po)=4,
    bcz(bcp+pz)=2 banks, so cross-batch WAR edges never put next-batch
    GEMMs behind this batch's normalize.
  * e/v/ocT/wp in bf16 (1 cyc/row at any N, cheap gpsimd masks); zq/zk stay
    f32r because bf16 strided evictions are 2x slower on DVE.
  * e1 tiles' left half is permanently zero (startup memset, exp writes only
    cols 128:256): the s-half-1 block is fully masked for t<128.
  * output projection packs head-pairs on 128 partitions (K=128): 6 matmuls.
"""

import sys

if "/opt/trn_rl_repo" not in sys.path:
    sys.path.insert(0, "/opt/trn_rl_repo")

import numpy as np
import ml_dtypes

import concourse.bass as bass
import concourse.mybir as mybir
import concourse.tile as tile
from concourse import bacc
from concourse.bass_utils import run_bass_kernel_spmd

F32 = mybir.dt.float32
F32R = mybir.dt.float32r
BF16 = mybir.dt.bfloat16
Exp = mybir.ActivationFunctionType.Exp

N_CORES = 8
B, T, C = 128, 256, 384
H, HS = 6, 64
NB = B // N_CORES          # batches per core
NP = NB // 2               # batch pairs per core
SCALE = C ** (-0.5)


def build_program(trace_sim=False, sim_init=False):
    nc = bacc.Bacc("TRN2", target_bir_lowering=False, debug=False)

    xT_d = nc.dram_tensor("xT", [NB, 3, 128, T], F32R, kind="ExternalInput")
    wq_d = nc.dram_tensor("wq", [C, C], F32R, kind="ExternalInput")
    wk_d = nc.dram_tensor("wk", [C, C], F32R, kind="ExternalInput")
    wv_d = nc.dram_tensor("wv", [C, C], F32R, kind="ExternalInput")
    # Wp pre-packed on host: wp2[64*par + d, jp, c] = Wp[(2*jp+par)*64 + d, c]
    wp_d = nc.dram_tensor("wp2", [128, 3, C], BF16, kind="ExternalInput")
    bpb_d = nc.dram_tensor("bpb", [128, C], F32, kind="ExternalInput")
    tri_d = nc.dram_tensor("tri", [128, 128], BF16, kind="ExternalInput")
    sel_d = nc.dram_tensor("sel6", [6, C], BF16, kind="ExternalInput")
    vsc_d = nc.dram_tensor("vsc", [NB, T, C], BF16)
    out_d = nc.dram_tensor("out", [NB, T, C], F32, kind="ExternalOutput")

    with tile.TileContext(nc, trace_sim=trace_sim) as tc:
        with (
            tc.tile_pool(name="const", bufs=1) as cst,
            tc.tile_pool(name="xt", bufs=3) as xtp,
            tc.tile_pool(name="zqk", bufs=2) as zqkp,
            tc.tile_pool(name="yv", bufs=4) as yvp,
            tc.tile_pool(name="vsb", bufs=3) as vp,
            tc.tile_pool(name="ee", bufs=6) as ep,
            tc.tile_pool(name="o65", bufs=14) as op65,
            tc.tile_pool(name="rs", bufs=2) as rsp,
            tc.tile_pool(name="oct", bufs=2) as octp,
            tc.tile_pool(name="zo", bufs=3) as zop,
            tc.tile_pool(name="psG", bufs=2, space="PSUM") as psG,
            tc.tile_pool(name="psA", bufs=4, space="PSUM") as psA,
            tc.tile_pool(name="psZ", bufs=2, space="PSUM") as psZ,
        ):
            # ---- constants (Q/K weights stream first so the prologue
            # GEMMs can start while the rest of the constants load) ----
            wq_sb = cst.tile([128, 3, C], F32R, tag="wq")
            wk_sb = cst.tile([128, 3, C], F32R, tag="wk")
            wv_sb = cst.tile([128, 3, C], F32R, tag="wv")
            nc.sync.dma_start(
                wq_sb[:], wq_d.rearrange("(k p) c -> p k c", p=128))
            wp_sb = cst.tile([128, 3, C], BF16, tag="wp")
            bpb = cst.tile([128, C], F32, tag="bpb")
            tri = cst.tile([128, 128], BF16, tag="tri")
            sel6 = cst.tile([6, C], BF16, tag="sel6")

            def load_tail_consts():
                nc.sync.dma_start(
                    wv_sb[:], wv_d.rearrange("(k p) c -> p k c", p=128))
                nc.sync.dma_start(wp_sb[:], wp_d[:])
                nc.sync.dma_start(bpb[:], bpb_d[:])
                nc.sync.dma_start(tri[:], tri_d[:])
                nc.sync.dma_start(sel6[:], sel_d[:])

            # Pre-zero the e1 ring (left half stays zero forever: exp only
            # writes cols 128:256) and pre-set the V ones columns per slot.
            for _ in range(6):
                e1i = ep.tile([128, T], BF16, tag="e1")
                nc.vector.memset(e1i[:], 0.0)
            for _ in range(3):
                v_ini = vp.tile([128, 12 * (HS + 1)], BF16, tag="vsb")
                if sim_init:
                    nc.vector.memset(v_ini[:], 0.0)
                nc.vector.memset(
                    v_ini[:].rearrange("p (g d) -> p g d", d=HS + 1)[:, :, HS], 1.0)

            # ---------------- pipeline building blocks ----------------
            xts = {}
            zqks = {}
            vsbs = {}

            def load_xt(p):
                xt = xtp.tile([128, 3, 2 * T], F32R, tag="xt")
                for n in range(2):
                    nc.sync.dma_start(
                        xt[:].rearrange("p k (n t) -> p k n t", n=2)[:, :, n, :],
                        xT_d[2 * p + n].rearrange("k p t -> p k t"))
                xts[p] = xt

            def qk_group(p, wi, m):
                # one (weight, m-tile) slice of a batch pair's Q/K GEMMs
                if p not in zqks:
                    zq2 = zqkp.tile([64, 12 * T], F32R, tag="zq")
                    zk2 = zqkp.tile([64, 12 * T], F32R, tag="zk")
                    if sim_init:
                        nc.vector.memset(zq2[:], 0.0)
                        nc.vector.memset(zk2[:], 0.0)
                    zqks[p] = (zq2, zk2)
                xt = xts[p]
                wsb = (wq_sb, wk_sb)[wi]
                z2 = zqks[p][wi]
                pq = psG.tile([128, 2 * T], F32, tag="gemm")
                for k in range(3):
                    nc.tensor.matmul(
                        pq[:],
                        wsb[:, k, m * 128:(m + 1) * 128],
                        xt[:, k, :],
                        start=(k == 0), stop=(k == 2),
                    )
                # eviction: [d, bb*1536 + 6t + j] for bb in {0,1}
                nc.vector.tensor_copy(
                    z2[:].rearrange("d (n f) -> d n f", n=2)
                         [:, :, 2 * m:6 * T:6]
                         .rearrange("d n t -> d (n t)"),
                    pq[0:64, :])
                nc.scalar.copy(
                    z2[:].rearrange("d (n f) -> d n f", n=2)
                         [:, :, 2 * m + 1:6 * T:6]
                         .rearrange("d n t -> d (n t)"),
                    pq[64:128, :])

            def qk_pair(p):
                for wi in range(2):
                    for m in range(3):
                        qk_group(p, wi, m)

            def v_group(b, m):
                xt = xts[b // 2]
                pv = psG.tile([128, C], F32, tag="gemm")
                for k in range(3):
                    nc.tensor.matmul(
                        pv[:],
                        xt[:, k, (b % 2) * T + m * 128:
                           (b % 2) * T + (m + 1) * 128],
                        wv_sb[:, k, :],
                        start=(k == 0), stop=(k == 2),
                    )
                yv = yvp.tile([128, C], BF16, tag="yv")
                nc.vector.tensor_copy(yv[:], pv[:])
                nc.sync.dma_start(vsc_d[b, m * 128:(m + 1) * 128, :], yv[:])
                if m == 1:
                    # V in chunk-row layout [128, 12*(64+1)]; ones cols preset
                    v_sb = vp.tile([128, 12 * (HS + 1)], BF16, tag="vsb")
                    nc.sync.dma_start(
                        v_sb[:].rearrange("p (g d) -> p g d", d=HS + 1)
                              [:, :, 0:HS],
                        vsc_d[b].rearrange("t c -> (t c)")
                                .rearrange("(g p d) -> p g d", p=128, d=64),
                    )
                    vsbs[b] = v_sb

            def v_batch(b):
                v_group(b, 0)
                v_group(b, 1)

            def attention(b, fillers=()):
                zq2, zk2 = zqks[b // 2]
                zq = zq2[:, (b % 2) * 6 * T:(b % 2 + 1) * 6 * T]
                zk = zk2[:, (b % 2) * 6 * T:(b % 2 + 1) * 6 * T]
                v_sb = vsbs.pop(b)
                rs6 = rsp.tile([6, T], BF16, tag="rs6")
                o65_tiles = []

                def s_head0(h):
                    # s-half 0: mask is upper-tri on cols 0:128, open after
                    st0 = psA.tile([128, T], F32, tag="att")
                    nc.tensor.matmul(
                        st0[:], zk[:, h * T:h * T + 128],
                        zq[:, h * T:(h + 1) * T],
                        start=True, stop=True)
                    e0 = ep.tile([128, T], BF16, tag="e0")
                    nc.scalar.activation(e0[:], st0[:], Exp,
                                         scale=SCALE)
                    nc.gpsimd.tensor_mul(e0[:, 0:128], e0[:, 0:128], tri[:])
                    return e0

                def s_head1(h):
                    # s-half 1: t<128 fully masked (e1 left half stays zero);
                    # cols 0:128 of st1 are computed but never read (f32r
                    # matmuls need free dim >= 256 for full rate anyway)
                    st1 = psA.tile([128, T], F32, tag="att")
                    nc.tensor.matmul(
                        st1[:], zk[:, h * T + 128:h * T + 256],
                        zq[:, h * T:(h + 1) * T],
                        start=True, stop=True)
                    e1 = ep.tile([128, T], BF16, tag="e1")
                    nc.scalar.activation(e1[:, 128:256], st1[:, 128:256], Exp,
                                         scale=SCALE)
                    nc.gpsimd.tensor_mul(e1[:, 128:256], e1[:, 128:256],
                                         tri[:])
                    return e1

                # S runs one head ahead of PV, interleaved S/PV so each f32r
                # S weight load hides under a PV stream and each head's
                # exp->mask chain drains before its PV issues
                e0s = [s_head0(0)]
                e1s = [s_head1(0)]
                for h in range(H):
                    if h + 1 < H:
                        e0s.append(s_head0(h + 1))
                    # PV with fused rowsum (row 64 of each lhsT chunk is ones)
                    po = psA.tile([HS + 1, T], F32, tag="att")
                    nc.tensor.matmul(
                        po[:],
                        v_sb[:, (2 * h) * (HS + 1):(2 * h + 1) * (HS + 1)],
                        e0s[h][:], start=True, stop=False)
                    if h + 1 < H:
                        e1s.append(s_head1(h + 1))
                    nc.tensor.matmul(
                        po[:],
                        v_sb[:, (2 * h + 1) * (HS + 1):(2 * h + 2) * (HS + 1)],
                        e1s[h][:], start=False, stop=True)
                    if h < len(fillers):
                        fillers[h]()
                    # evict now (frees the PSUM slot): rows 0:64 = O^T, row 64
                    # = softmax denominator
                    o65 = op65.tile([HS + 1, T], BF16, tag="o65")
                    nc.scalar.copy(o65[:], po[:])
                    o65_tiles.append(o65)
                    # gather this head's denominators into rs6 row h (DMA:
                    # engine copies cannot write partitions not 32-aligned)
                    nc.sync.dma_start(rs6[h:h + 1, :], o65[HS:HS + 1, :])
                for f in fillers[H:]:
                    f()
                return rs6, o65_tiles

            def norm_pieces(b, rs6, o65_tiles):
                # normalize + projection, decomposed into five tensor-work
                # chunks so they can double as fillers for the last batch
                st = {}

                def bc_piece(hp):
                    if "recip6" not in st:
                        recip6 = rsp.tile([6, T], BF16, tag="recip6")
                        with nc.allow_low_precision(
                                reason="softmax denom in bf16"):
                            nc.vector.reciprocal(recip6[:], rs6[:])
                        # ocT2[64*(g%2) + d, g//2] = O^T[d, g] / denom[g]
                        st["recip6"] = recip6
                        st["ocT2"] = octp.tile([128, 768], BF16, tag="ocT2",
                                               name="ocT2")
                    # bc broadcasts a head PAIR per matmul: out rows 0:64 =
                    # head 2hp's denominators, rows 64:128 = head 2hp+1's
                    bcp = psZ.tile([128, T], F32, tag="bcz")
                    nc.tensor.matmul(
                        bcp[:], sel6[:, 128 * hp:128 * (hp + 1)],
                        st["recip6"][:], start=True, stop=True)
                    for hh in range(2):
                        h = 2 * hp + hh
                        o65 = o65_tiles[h]
                        for par in range(2):
                            nc.vector.tensor_mul(
                                st["ocT2"][64 * par:64 * (par + 1),
                                           128 * h:128 * (h + 1)],
                                o65[0:HS, par::2],
                                bcp[64 * hh:64 * (hh + 1), par::2])

                def proj_piece(m):
                    pz = psZ.tile([128, C], F32, tag="bcz")
                    for jp in range(3):
                        nc.tensor.matmul(
                            pz[:],
                            st["ocT2"][:, 384 * m + jp:384 * (m + 1):3],
                            wp_sb[:, jp, :],
                            start=(jp == 0), stop=(jp == 2),
                        )
                    zo = zop.tile([128, C], F32, tag="zo")
                    nc.vector.tensor_add(zo[:], pz[:], bpb[:])
                    nc.sync.dma_start(out_d[b, m * 128:(m + 1) * 128, :], zo[:])

                return ([lambda hp=hp: bc_piece(hp) for hp in range(H // 2)]
                        + [lambda m=m: proj_piece(m) for m in range(2)])

            def norm_proj(b, rs6, o65_tiles):
                for f in norm_pieces(b, rs6, o65_tiles):
                    f()

            # ---------------- software-pipelined schedule ----------------
            # Gap work (next-batch GEMMs) sits between PV(b) and the
            # normalize-dependent bc/proj matmuls so the tensor queue never
            # drains while the scalar->DMA->DVE reciprocal round trip runs.
            load_xt(0)
            nc.sync.dma_start(
                wk_sb[:], wk_d.rearrange("(k p) c -> p k c", p=128))
            load_tail_consts()
            qk_pair(0)
            v_batch(0)
            v_batch(1)
            load_xt(1)
            prev = None
            for b in range(NB):
                fillers = []
                if b % 2 == 0:
                    if b // 2 + 1 < NP:
                        p = b // 2 + 1
                        for wi in range(2):
                            for m in range(3):
                                fillers.append(
                                    lambda p=p, wi=wi, m=m: qk_group(p, wi, m))
                else:
                    for bn in (b + 1, b + 2):
                        if bn < NB:
                            for m in range(2):
                                fillers.append(
                                    lambda bn=bn, m=m: v_group(bn, m))
                    if b // 2 + 2 < NP:
                        fillers.append(lambda p=b // 2 + 2: load_xt(p))
                # normalize+project run one batch behind: by the time bc(b-1)
                # reaches the tensor queue, recip(b-1) completed under
                # attention(b)'s shadow. For the last batch (no next-batch
                # GEMMs left) the previous batch's norm/proj pieces are the
                # fillers; slots 0-1 stay empty so recip(b-1) has time.
                if b == NB - 1 and prev is not None:
                    fillers = [lambda: None, lambda: None]
                    fillers += norm_pieces(b - 1, *prev)
                    prev = None
                cur = attention(b, fillers)
                if prev is not None:
                    norm_proj(b - 1, *prev)
                prev = cur
            norm_proj(NB - 1, *prev)

    nc.compile()
    return nc


def make_in_maps(x, Wk, Wq, Wv, Wp, bp):
    ut = (np.arange(128)[:, None] <= np.arange(128)[None, :])
    tri = ut.astype(ml_dtypes.bfloat16)
    sel6 = np.zeros((6, C), np.float32)
    for h in range(6):
        sel6[h, 64 * h:64 * (h + 1)] = 1.0
    wp2 = (np.asarray(Wp, np.float32).reshape(3, 2, 64, C)
           .transpose(1, 2, 0, 3).reshape(128, 3, C))
    common = dict(
        wq=np.ascontiguousarray(Wq, np.float32),
        wk=np.ascontiguousarray(Wk, np.float32),
        wv=np.ascontiguousarray(Wv, np.float32),
        wp2=np.ascontiguousarray(wp2).astype(ml_dtypes.bfloat16),
        bpb=np.broadcast_to(np.asarray(bp, np.float32), (128, C)).copy(),
        tri=np.ascontiguousarray(tri),
        sel6=sel6.astype(ml_dtypes.bfloat16),
    )
    in_maps = []
    for c in range(N_CORES):
        xs = np.asarray(x[c * NB:(c + 1) * NB], np.float32)
        xT = np.ascontiguousarray(xs.transpose(0, 2, 1)).reshape(NB, 3, 128, T)
        in_maps.append(dict(common, xT=xT))
    return in_maps


_CACHE = {}


def kernel(x, Wk, Wq, Wv, Wp, bp, _trace=False, _tmpdir=None):
    if "nc" not in _CACHE:
        _CACHE["nc"] = build_program()
    nc = _CACHE["nc"]
    in_maps = make_in_maps(x, Wk, Wq, Wv, Wp, bp)
    res = run_bass_kernel_spmd(nc, in_maps, list(range(N_CORES)),
                               trace=_trace, tmpdir=_tmpdir)
    _CACHE["last_results"] = res
    out = np.concatenate([np.asarray(r["out"]) for r in res.results], axis=0)
    return out.reshape(B, T, C).astype(np.float32)
